# revision 5
# baseline (speedup 1.0000x reference)
"""Trainium2 Bass kernel for the 16-head MHA problem (B=4, S=2048, D=1024).

The reference adds mask*2^32 to the raw scores BEFORE the 1/sqrt(dk) scale
and softmax.  In fp32 the masked softmax collapses exactly to
indicator/row_count (see git history / derivation in the original kernel):
P = indicator/count is identical for all 16 heads, and with G = Wv @ Wo

    out[b] = (P @ values[b]) @ G + (bv @ Wo + bo)

P @ x is a suffix-mean.  NEW STRUCTURE (v2, "pre-scan"): since prefix-sum
and the G-projection commute, the VALUES are scanned FIRST:

    host packs, per core, v_packed = [0-row; reversed-half[:-1]] so that an
    INCLUSIVE prefix scan along q' gives S[q'] = sum of the suffix rows,
    with the beyond-core carry supplied as the scan initial.
    DVE scans v_packed chunks (fp16, f32 state), Pool scales each scanned
    chunk by 1/count (per column), and the PE GEMM

        psum[jt, q'] = sum_j G[j, jt-cols] * (S[q', j]/c[q'])

    then directly produces the FINAL output tile: the post-GEMM path is a
    single Scalar/DVE fp16 copy out of PSUM + DMA.  No scan and no scale on
    the output side, so PSUM banks recycle fast and the kernel tail is
    copy+DMA only.

Phase structure per core: phase 1 = q' columns 0..511 with k-outer /
jt-inner over 8 PSUM banks (so compute starts as soon as the first
128-contraction chunk of weights+scanned-values is ready); phase 2 = q'
512..1023 the same, with the last three k-groups emitted jt-major so tile
completions stagger and the copies/DMA drain in parallel with compute.
Dummy matmuls warm the PE (HAM) from kernel start until real data lands.

Sharding: 8 cores = 4 batches x 2 sequence halves; each core owns 1024
output rows exclusively.  Rows with no unmasked entry (only the global
last row) get a true softmax, patched on the host from the raw inputs.
"""

import numpy as np

import concourse.bass as bass
import concourse.mybir as mybir
import concourse.tile as tile
from concourse import bacc, bass_utils

# ---------------------------------------------------------------- constants
B, S, D = 4, 2048, 1024
HEADS, DK = 16, 64
N_CORES = 8
SH = S // 2                 # 1024 output rows (q') per core
NK = D // 128               # 8 contraction chunks (values columns)
NJT = D // 128              # 8 output-row (d_out) tiles
NQC = 2                     # two 512-wide q' column phases
CW = 512
MASK_CONST = np.float32(4294967296.0)   # +2^32, faithful to the reference
SCALE = 1.0 / np.sqrt(np.float32(DK))   # 1/8

F32 = mybir.dt.float32
FP16 = mybir.dt.float16
ALU = mybir.AluOpType


# ------------------------------------------------------------- kernel build
def _build():
    nc = bacc.Bacc("TRN2", target_bir_lowering=False, debug=False,
                   num_devices=N_CORES)

    def din(name, shape, dt):
        return nc.dram_tensor(name, shape, dt, kind="ExternalInput").ap()

    # g[k][p, jt, j] = G[k*128+p, jt*128+j]
    g = din("g", (NK, 128, NJT, 128), FP16)
    # vt[k][p, q'] = v_packed[q', k*128+p]  (both q' halves in one chunk)
    vt = din("vt", (NK, 128, SH), FP16)
    # rvec[p, k] = beyond-core column sums (scan initial), f32
    rvec = din("rvec", (128, NK), F32)
    # sbc[j, q'] = 1/count broadcast over j (0 where count==0)
    sbcd = din("sbc", (128, SH), FP16)

    out = nc.dram_tensor("out", (D, SH), FP16, kind="ExternalOutput").ap()
    wout = nc.dram_tensor("wout", (128, 16), FP16, kind="ExternalOutput").ap()

    with tile.TileContext(nc) as tc:
        with (
            tc.tile_pool(name="res", bufs=1) as res,
            tc.tile_pool(name="cps", bufs=4) as cpool,
            tc.tile_pool(name="cps2", bufs=2) as cpool2,
            tc.tile_pool(name="acc", bufs=8, space="PSUM") as accp,
        ):
            g_sb = res.tile([128, NK, NJT, 128], FP16, tag="g")
            vt_sb = res.tile([128, NK, SH], FP16, tag="vt")
            svt = res.tile([128, NK, SH], FP16, tag="svt")     # scanned
            svs = res.tile([128, NK, SH], FP16, tag="svs")     # scanned*inv
            rvec_sb = res.tile([128, NK], F32, tag="rvec")
            sbc = res.tile([128, SH], FP16, tag="sbc")
            zeros = res.tile([128, CW], FP16, tag="zeros")
            scr = res.tile([128, 256], FP16, tag="scr")
            warm = res.tile([128, 16], FP16, tag="warm")

            # ---- Pool: constants needed by the scan/dummy paths
            nc.gpsimd.memset(zeros[:], 0.0)
            nc.gpsimd.memset(scr[:], 0.125)

            # ---- Scalar queue: small inputs needed first
            nc.scalar.dma_start(rvec_sb[:], rvec[:])
            nc.scalar.dma_start(sbc[:], sbcd[:])

            # ---- Sync queue: interleave vt (scan feed) and g (weights)
            order = ["v0", "g0", "v1", "g1", "v2", "g2", "g3", "v3",
                     "g4", "v4", "g5", "v5", "g6", "v6", "g7", "v7"]
            for item in order:
                k = int(item[1])
                if item[0] == "v":
                    nc.sync.dma_start(vt_sb[:, k], vt[k])
                else:
                    nc.sync.dma_start(g_sb[:, k], g[k])

            # ---- PE warm-up (HAM) while first chunks land
            wps = accp.tile([128, CW], F32, tag="acc")
            for d in range(12):
                nc.tensor.matmul(wps[:, 0:256], scr[:, 0:128], scr[:],
                                 start=(d == 0), stop=(d == 11))
            nc.scalar.copy(warm[:], wps[:, 0:16])
            nc.scalar.dma_start(wout[:], warm[:])

            # ---- DVE scans + Pool scales, in consumption order.
            # (k, qc) chunk: scan v_packed -> svt, scale by 1/c -> svs.
            def emit_scan(k, qc, sub=1):
                w = CW // sub
                for i in range(sub):
                    lo = qc * CW + i * w
                    if lo == 0:
                        init = rvec_sb[:, k:k + 1]
                    else:
                        init = svt[:, k, lo - 1:lo]
                    nc.vector.tensor_tensor_scan(
                        svt[:, k, lo:lo + w], zeros[:, 0:w],
                        vt_sb[:, k, lo:lo + w], init, ALU.add, ALU.add)
                    nc.gpsimd.tensor_mul(
                        svs[:, k, lo:lo + w], svt[:, k, lo:lo + w],
                        sbc[:, lo:lo + w])

            emit_scan(0, 0, sub=4)           # fine-grained: shortest latency
            for k in range(1, NK):
                emit_scan(k, 0)
            for k in range(NK):
                emit_scan(k, 1)

            # ---- GEMM phases.  k-outer / jt-inner over 8 PSUM banks; the
            # last 3 k-groups run jt-major so tile completions stagger.
            banks = {}

            def mm(k, jt, qc):
                if k == 0:
                    bank_t = accp.tile([128, CW], F32, tag="acc")
                    banks[jt] = bank_t
                nc.tensor.matmul(banks[jt][:], g_sb[:, k, jt],
                                 svs[:, k, qc * CW:(qc + 1) * CW],
                                 start=(k == 0), stop=(k == NK - 1))

            def emit_phase(qc, finish):
                for k in range(NK - 3):
                    for jt in range(NJT):
                        mm(k, jt, qc)
                for jt in range(NJT):
                    for k in range(NK - 3, NK):
                        mm(k, jt, qc)
                    finish(jt, banks[jt])

            # phase 1: copies on Scalar, output kicks on Sync
            def fin1(jt, bank):
                cp = cpool.tile([128, CW], FP16, tag="c")
                nc.scalar.copy(cp[:], bank[:])
                nc.sync.dma_start(out[jt * 128:(jt + 1) * 128, 0:CW], cp[:])

            emit_phase(0, fin1)

            # phase 2: copies alternate DVE/Scalar, kicks alternate
            # Sync/Pool; the very last tile goes as two parallel halves.
            def fin2(jt, bank):
                if jt == NJT - 1:
                    for half in range(2):
                        cp = cpool2.tile([128, 256], FP16, tag="c2")
                        lo = half * 256
                        if half == 0:
                            nc.scalar.copy(cp[:], bank[:, lo:lo + 256])
                            nc.sync.dma_start(
                                out[jt * 128:(jt + 1) * 128,
                                    CW + lo:CW + lo + 256], cp[:])
                        else:
                            nc.vector.tensor_copy(cp[:], bank[:, lo:lo + 256])
                            nc.gpsimd.dma_start(
                                out[jt * 128:(jt + 1) * 128,
                                    CW + lo:CW + lo + 256], cp[:])
                    return
                cp = cpool.tile([128, CW], FP16, tag="c")
                if jt % 2 == 0:
                    nc.vector.tensor_copy(cp[:], bank[:])
                else:
                    nc.scalar.copy(cp[:], bank[:])
                eng = nc.sync if jt % 2 else nc.gpsimd
                eng.dma_start(out[jt * 128:(jt + 1) * 128, CW:SH], cp[:])

            emit_phase(1, fin2)

    nc.compile()
    return nc


# ------------------------------------------------------------- host wrapper
_CACHE: dict = {}
LAST_RESULTS = None
LAST_IN_MAPS = None


def _get_kernel():
    if "k" not in _CACHE:
        _CACHE["k"] = _build()
    return _CACHE["k"]


def _host_fallback(values, mask2d, G, row_bias, out):
    """Generic-mask path (never hit for the causal-complement mask):
    P = indicator/row_count computed densely on the host."""
    ind = ((mask2d * MASK_CONST) == MASK_CONST).astype(np.float32)
    cnt = ind.sum(axis=1)
    ok = cnt > 0
    P = ind[ok] / cnt[ok, None]
    for b in range(B):
        out[b][ok] = (P @ values[b]) @ G + row_bias


def kernel(queries, keys, values, mask, Wq, bq, Wk, bk, Wv, bv, Wo, bo):
    queries = np.asarray(queries, dtype=np.float32)
    keys = np.asarray(keys, dtype=np.float32)
    values = np.asarray(values, dtype=np.float32)
    mask2d = np.ascontiguousarray(
        np.asarray(mask, dtype=np.float32).reshape(S, S))
    Wq = np.asarray(Wq, dtype=np.float32); bq_ = np.asarray(bq, dtype=np.float32)
    Wk = np.asarray(Wk, dtype=np.float32); bk_ = np.asarray(bk, dtype=np.float32)
    Wv = np.asarray(Wv, dtype=np.float32); bv_ = np.asarray(bv, dtype=np.float32)
    Wo = np.asarray(Wo, dtype=np.float32); bo_ = np.asarray(bo, dtype=np.float32)

    G = Wv @ Wo                                  # (D, D) fp32
    row_bias = bv_ @ Wo + bo_                    # (D,)

    ind = ((mask2d * MASK_CONST) == MASK_CONST)
    qfix = np.where(~ind.any(axis=1))[0]
    causal = np.array_equal(
        ind, np.triu(np.ones((S, S), dtype=bool), k=1))

    out = np.empty((B, S, D), dtype=np.float32)

    if causal:
        nc = _get_kernel()

        g_host = np.ascontiguousarray(
            G.astype(np.float16).reshape(NK, 128, NJT, 128))

        in_maps = []
        for core in range(N_CORES):
            b, h = divmod(core, 2)
            v_half = values[b, h * SH:(h + 1) * SH]
            v_packed = np.vstack(
                [np.zeros((1, D), np.float32),
                 v_half[::-1][:SH - 1]]).astype(np.float16)
            # vt[k][p, q'] = v_packed[q', k*128+p]
            vt_host = np.ascontiguousarray(
                v_packed.reshape(SH, NK, 128).transpose(1, 2, 0))
            if h == 0:
                beyond = values[b, SH:, :].sum(axis=0, dtype=np.float64)
                rv = beyond.astype(np.float32)
            else:
                rv = np.zeros(D, np.float32)
            rvec_host = np.ascontiguousarray(rv.reshape(NK, 128).T)
            c = (1 - h) * SH + np.arange(SH, dtype=np.float64)
            if h == 1:
                c[0] = 1.0
            inv = (1.0 / c).astype(np.float16)
            if h == 1:
                inv[0] = 0.0
            sbc_host = np.ascontiguousarray(
                np.broadcast_to(inv, (128, SH)))
            in_maps.append({
                "g": g_host,
                "vt": vt_host,
                "rvec": rvec_host,
                "sbc": sbc_host,
            })

        res = bass_utils.run_bass_kernel_spmd(
            nc, in_maps, core_ids=list(range(N_CORES)))

        global LAST_RESULTS, LAST_IN_MAPS
        LAST_RESULTS = res
        LAST_IN_MAPS = in_maps

        for core in range(N_CORES):
            b, h = divmod(core, 2)
            # out dram is [d_out, q'] with q' reversed: undo both
            o = res.results[core]["out"].astype(np.float32).T[::-1, :]
            out[b, h * SH:(h + 1) * SH, :] = o + row_bias
    else:
        _host_fallback(values, mask2d, G, row_bias, out)

    # ---------------- host patch for rows with no indicator entry
    # True softmax for these rows, by reassociation so neither Q nor K is
    # ever materialized: s = ((q Wq) Wk^T) keys^T; pure fp32 numpy.
    if len(qfix) > 0:
        q = qfix
        mrow = mask2d[q] * MASK_CONST                       # [nq, S]
        for b in range(B):
            Qr = queries[b][q] @ Wq + bq_                   # [nq, HEADS*DK]
            Oc = np.empty((len(q), HEADS * DK), dtype=np.float32)
            for H in range(HEADS):
                hs = slice(H * DK, (H + 1) * DK)
                t = Qr[:, hs] @ Wk[:, hs].T                 # [nq, D]
                scr = t @ keys[b].T                         # [nq, S]
                scr = scr + (Qr[:, hs] @ bk_[hs])[:, None]  # K-bias term
                y = (scr + mrow) * np.float32(SCALE)
                y = y - y.max(axis=1, keepdims=True)
                e = np.exp(y, dtype=np.float32)
                p = (e / e.sum(axis=1, keepdims=True)).astype(np.float32)
                z = p @ values[b]                           # [nq, D]
                Oc[:, hs] = z @ Wv[:, hs] + bv_[hs]
            out[b][q] = Oc @ Wo + bo_
    return out.reshape(B, S, D)


# revision 6
# speedup vs baseline: 1.0765x; 1.0765x over previous
"""Trainium2 Bass kernel for the 16-head MHA problem (B=4, S=2048, D=1024).

The reference adds mask*2^32 to the raw scores BEFORE the 1/sqrt(dk) scale
and softmax.  In fp32 the masked softmax collapses exactly to
indicator/row_count: P = indicator/count is identical for all 16 heads,
and with G = Wv @ Wo precomputed from the weight inputs

    out[b] = (P @ values[b]) @ G + (bv @ Wo + bo)

P @ x is a suffix-mean.  Since prefix-sum and the G-projection commute,
the VALUES are scanned FIRST: the host packs, per core, v_packed =
[0-row; reversed-half[:-1]] so an INCLUSIVE prefix scan along q' gives
S[q'] = sum of the suffix rows, with the beyond-core carry as the scan
initial.  DVE does ONLY these 16 fp16 scans (f32 state).  The GEMM runs
TRANSPOSED, with q' on the PSUM partition axis:

    psum[q', d] = sum_j S[q', j] * G[j, d]
      stationary = svt[:, k, q'-block]  (scanned values, 128x128)
      moving     = g[:, k, d-half]      (512 wide)

so the 1/count scale is per-PARTITION and fuses into the Scalar engine's
PSUM->SBUF fp16 copy (activation Copy with a [128,1] scale).  The output
path per tile is that single Scalar op + a DMA: no Pool streaming, no
DVE on the output path, hence no SBUF port contention (Pool and DVE
share SBUF ports - concurrent streaming halves both).

Phases: q' 0..511 then 512..1023, k-outer / tile-inner over 8 PSUM banks
so compute starts as soon as the first scan chunk and weight chunk land;
the last three k-groups are emitted tile-major so completions stagger
and copies/DMA drain during compute.  Dummy matmuls warm the PE (HAM)
from kernel start until real data arrives.

Sharding: 8 cores = 4 batches x 2 sequence halves; each core owns 1024
output rows exclusively.  Rows with no unmasked entry (only the global
last row) get a true softmax, patched on the host from the raw inputs.
"""

import numpy as np

import concourse.bass as bass
import concourse.mybir as mybir
import concourse.tile as tile
from concourse import bacc, bass_utils

# ---------------------------------------------------------------- constants
B, S, D = 4, 2048, 1024
HEADS, DK = 16, 64
N_CORES = 8
SH = S // 2                 # 1024 output rows (q') per core
NK = D // 128               # 8 contraction chunks (values columns)
NQT = SH // 128             # 8 q' row tiles
CW = 512
MASK_CONST = np.float32(4294967296.0)   # +2^32, faithful to the reference
SCALE = 1.0 / np.sqrt(np.float32(DK))   # 1/8

F32 = mybir.dt.float32
FP16 = mybir.dt.float16
ALU = mybir.AluOpType
ACTF = mybir.ActivationFunctionType


# ------------------------------------------------------------- kernel build
def _build():
    nc = bacc.Bacc("TRN2", target_bir_lowering=False, debug=False,
                   num_devices=N_CORES)

    def din(name, shape, dt):
        return nc.dram_tensor(name, shape, dt, kind="ExternalInput").ap()

    # g[k][p, d] = G[k*128+p, d]
    g = din("g", (NK, 128, D), FP16)
    # vt[k][p, q'] = v_packed[q', k*128+p]  (both q' halves in one chunk)
    vt = din("vt", (NK, 128, SH), FP16)
    # rvec[p, k] = beyond-core column sums (scan initial), f32
    rvec = din("rvec", (128, NK), F32)
    # inv8[p, t] = 1/count for q' = t*128+p (0 where count==0), f32
    inv8d = din("inv8", (128, NQT), F32)

    out = nc.dram_tensor("out", (SH, D), FP16, kind="ExternalOutput").ap()
    wout = nc.dram_tensor("wout", (128, 16), FP16, kind="ExternalOutput").ap()

    with tile.TileContext(nc) as tc:
        with (
            tc.tile_pool(name="res", bufs=1) as res,
            tc.tile_pool(name="cps", bufs=4) as cpool,
            tc.tile_pool(name="cps2", bufs=2) as cpool2,
            tc.tile_pool(name="acc", bufs=8, space="PSUM") as accp,
        ):
            g_sb = res.tile([128, NK, D], FP16, tag="g")
            vt_sb = res.tile([128, NK, SH], FP16, tag="vt")
            svt = res.tile([128, NK, SH], FP16, tag="svt")     # scanned
            rvec_sb = res.tile([128, NK], F32, tag="rvec")
            inv8 = res.tile([128, NQT], F32, tag="inv8")
            zeros = res.tile([128, CW], FP16, tag="zeros")
            scr = res.tile([128, 256], FP16, tag="scr")
            warm = res.tile([128, 16], FP16, tag="warm")

            # ---- Pool: constants needed by the scan/dummy paths
            nc.gpsimd.memset(zeros[:], 0.0)
            nc.gpsimd.memset(scr[:], 0.125)

            # ---- Scalar queue: small inputs needed first
            nc.scalar.dma_start(rvec_sb[:], rvec[:])
            nc.scalar.dma_start(inv8[:], inv8d[:])

            # ---- Sync queue: interleave vt (scan feed) and g (weights)
            order = ["v0", "g0", "v1", "g1", "v2", "g2", "g3", "v3",
                     "g4", "v4", "g5", "v5", "g6", "v6", "g7", "v7"]
            for item in order:
                k = int(item[1])
                if item[0] == "v":
                    nc.sync.dma_start(vt_sb[:, k], vt[k])
                else:
                    nc.sync.dma_start(g_sb[:, k], g[k])

            # ---- PE warm-up (HAM) while first chunks land
            wps = accp.tile([128, CW], F32, tag="acc")
            for d in range(9):
                nc.tensor.matmul(wps[:, 0:256], scr[:, 0:128], scr[:],
                                 start=(d == 0), stop=(d == 8))
            nc.scalar.copy(warm[:], wps[:, 0:16])
            nc.scalar.dma_start(wout[:], warm[:])

            # ---- DVE scans, in consumption order; first chunk split fine
            def emit_scan(k, qc, sub=1):
                w = CW // sub
                for i in range(sub):
                    lo = qc * CW + i * w
                    if lo == 0:
                        init = rvec_sb[:, k:k + 1]
                    else:
                        init = svt[:, k, lo - 1:lo]
                    nc.vector.tensor_tensor_scan(
                        svt[:, k, lo:lo + w], zeros[:, 0:w],
                        vt_sb[:, k, lo:lo + w], init, ALU.add, ALU.add)

            emit_scan(0, 0, sub=4)           # fine-grained: shortest latency
            for k in range(1, NK):
                emit_scan(k, 0)
            for k in range(NK):
                emit_scan(k, 1)

            # ---- GEMM phases.  k-outer / tile-inner over 8 PSUM banks;
            # the last 3 k-groups run tile-major so completions stagger.
            banks = {}

            def mm(k, qt, dh):
                if k == 0:
                    bank_t = accp.tile([128, CW], F32, tag="acc")
                    banks[(qt, dh)] = bank_t
                nc.tensor.matmul(banks[(qt, dh)][:],
                                 svt[:, k, qt * 128:(qt + 1) * 128],
                                 g_sb[:, k, dh * CW:(dh + 1) * CW],
                                 start=(k == 0), stop=(k == NK - 1))

            def emit_phase(phase, finish):
                tiles = [(4 * phase + i, dh) for i in range(4)
                         for dh in range(2)]
                for k in range(NK - 3):
                    for qt, dh in tiles:
                        mm(k, qt, dh)
                for qt, dh in tiles:
                    for k in range(NK - 3, NK):
                        mm(k, qt, dh)
                    finish(qt, dh, banks[(qt, dh)])

            def store(qt, dh, cp, deng, lo=0, w=CW):
                deng.dma_start(
                    out[qt * 128:(qt + 1) * 128,
                        dh * CW + lo:dh * CW + lo + w], cp[:])

            # phase 1: fused scale+copy on Scalar, output kicks on Sync
            def fin1(qt, dh, bank):
                cp = cpool.tile([128, CW], FP16, tag="c")
                nc.scalar.activation(cp[:], bank[:], ACTF.Copy,
                                     scale=inv8[:, qt:qt + 1])
                store(qt, dh, cp, nc.sync)

            emit_phase(0, fin1)

            # phase 2: copies alternate Scalar/DVE, kicks alternate
            # Sync/Pool; the very last tile goes as two parallel halves.
            n2 = [0]

            def fin2(qt, dh, bank):
                if (qt, dh) == (NQT - 1, 1):
                    for half in range(2):
                        cp = cpool2.tile([128, 256], FP16, tag="c2")
                        lo = half * 256
                        if half == 0:
                            nc.scalar.activation(
                                cp[:], bank[:, lo:lo + 256], ACTF.Copy,
                                scale=inv8[:, qt:qt + 1])
                            store(qt, dh, cp, nc.sync, lo, 256)
                        else:
                            nc.vector.tensor_scalar_mul(
                                cp[:], bank[:, lo:lo + 256],
                                inv8[:, qt:qt + 1])
                            store(qt, dh, cp, nc.gpsimd, lo, 256)
                    return
                cp = cpool.tile([128, CW], FP16, tag="c")
                if n2[0] % 2 == 0:
                    nc.scalar.activation(cp[:], bank[:], ACTF.Copy,
                                         scale=inv8[:, qt:qt + 1])
                    store(qt, dh, cp, nc.sync)
                else:
                    nc.vector.tensor_scalar_mul(cp[:], bank[:],
                                                inv8[:, qt:qt + 1])
                    store(qt, dh, cp, nc.gpsimd)
                n2[0] += 1

            emit_phase(1, fin2)

    nc.compile()
    return nc


# ------------------------------------------------------------- host wrapper
_CACHE: dict = {}
LAST_RESULTS = None
LAST_IN_MAPS = None


def _get_kernel():
    if "k" not in _CACHE:
        _CACHE["k"] = _build()
    return _CACHE["k"]


def _host_fallback(values, mask2d, G, row_bias, out):
    """Generic-mask path (never hit for the causal-complement mask):
    P = indicator/row_count computed densely on the host."""
    ind = ((mask2d * MASK_CONST) == MASK_CONST).astype(np.float32)
    cnt = ind.sum(axis=1)
    ok = cnt > 0
    P = ind[ok] / cnt[ok, None]
    for b in range(B):
        out[b][ok] = (P @ values[b]) @ G + row_bias


def kernel(queries, keys, values, mask, Wq, bq, Wk, bk, Wv, bv, Wo, bo):
    queries = np.asarray(queries, dtype=np.float32)
    keys = np.asarray(keys, dtype=np.float32)
    values = np.asarray(values, dtype=np.float32)
    mask2d = np.ascontiguousarray(
        np.asarray(mask, dtype=np.float32).reshape(S, S))
    Wq = np.asarray(Wq, dtype=np.float32); bq_ = np.asarray(bq, dtype=np.float32)
    Wk = np.asarray(Wk, dtype=np.float32); bk_ = np.asarray(bk, dtype=np.float32)
    Wv = np.asarray(Wv, dtype=np.float32); bv_ = np.asarray(bv, dtype=np.float32)
    Wo = np.asarray(Wo, dtype=np.float32); bo_ = np.asarray(bo, dtype=np.float32)

    G = Wv @ Wo                                  # (D, D) fp32
    row_bias = bv_ @ Wo + bo_                    # (D,)

    ind = ((mask2d * MASK_CONST) == MASK_CONST)
    qfix = np.where(~ind.any(axis=1))[0]
    causal = np.array_equal(
        ind, np.triu(np.ones((S, S), dtype=bool), k=1))

    out = np.empty((B, S, D), dtype=np.float32)

    if causal:
        nc = _get_kernel()

        g_host = np.ascontiguousarray(
            G.astype(np.float16).reshape(NK, 128, D))

        in_maps = []
        for core in range(N_CORES):
            b, h = divmod(core, 2)
            v_half = values[b, h * SH:(h + 1) * SH]
            v_packed = np.vstack(
                [np.zeros((1, D), np.float32),
                 v_half[::-1][:SH - 1]]).astype(np.float16)
            # vt[k][p, q'] = v_packed[q', k*128+p]
            vt_host = np.ascontiguousarray(
                v_packed.reshape(SH, NK, 128).transpose(1, 2, 0))
            if h == 0:
                beyond = values[b, SH:, :].sum(axis=0, dtype=np.float64)
                rv = beyond.astype(np.float32)
            else:
                rv = np.zeros(D, np.float32)
            rvec_host = np.ascontiguousarray(rv.reshape(NK, 128).T)
            c = (1 - h) * SH + np.arange(SH, dtype=np.float64)
            if h == 1:
                c[0] = 1.0
            inv = (1.0 / c).astype(np.float32)
            if h == 1:
                inv[0] = 0.0
            inv8_host = np.ascontiguousarray(inv.reshape(NQT, 128).T)
            in_maps.append({
                "g": g_host,
                "vt": vt_host,
                "rvec": rvec_host,
                "inv8": inv8_host,
            })

        res = bass_utils.run_bass_kernel_spmd(
            nc, in_maps, core_ids=list(range(N_CORES)))

        global LAST_RESULTS, LAST_IN_MAPS
        LAST_RESULTS = res
        LAST_IN_MAPS = in_maps

        for core in range(N_CORES):
            b, h = divmod(core, 2)
            # out dram is [q', d_out] with q' reversed: undo the reversal
            o = res.results[core]["out"].astype(np.float32)[::-1, :]
            out[b, h * SH:(h + 1) * SH, :] = o + row_bias
    else:
        _host_fallback(values, mask2d, G, row_bias, out)

    # ---------------- host patch for rows with no indicator entry
    # True softmax for these rows, by reassociation so neither Q nor K is
    # ever materialized: s = ((q Wq) Wk^T) keys^T; pure fp32 numpy.
    if len(qfix) > 0:
        q = qfix
        mrow = mask2d[q] * MASK_CONST                       # [nq, S]
        for b in range(B):
            Qr = queries[b][q] @ Wq + bq_                   # [nq, HEADS*DK]
            Oc = np.empty((len(q), HEADS * DK), dtype=np.float32)
            for H in range(HEADS):
                hs = slice(H * DK, (H + 1) * DK)
                t = Qr[:, hs] @ Wk[:, hs].T                 # [nq, D]
                scr = t @ keys[b].T                         # [nq, S]
                scr = scr + (Qr[:, hs] @ bk_[hs])[:, None]  # K-bias term
                y = (scr + mrow) * np.float32(SCALE)
                y = y - y.max(axis=1, keepdims=True)
                e = np.exp(y, dtype=np.float32)
                p = (e / e.sum(axis=1, keepdims=True)).astype(np.float32)
                z = p @ values[b]                           # [nq, D]
                Oc[:, hs] = z @ Wv[:, hs] + bv_[hs]
            out[b][q] = Oc @ Wo + bo_
    return out.reshape(B, S, D)


# revision 7
# speedup vs baseline: 1.3028x; 1.2102x over previous
"""Trainium2 Bass kernel for the 16-head MHA problem (B=4, S=2048, D=1024).

The reference adds mask*2^32 to the raw scores BEFORE the 1/sqrt(dk) scale
and softmax.  In fp32, for any row with at least one entry where
fl32(mask*2^32) == 2^32, the masked softmax collapses exactly to
indicator/row_count, identical for all 16 heads.  With G = Wv @ Wo
precomputed from the weight inputs the whole module factors:

    out[b] = (P @ values[b]) @ G + (bv @ Wo + bo)

For the causal-complement mask P@x is a suffix-mean.  Per core the device
work is a dense GEMM VG = values^T projected through G (output-transposed,
[d_out, seq] layout) plus a DVE prefix scan: the host packs the sequence
axis REVERSED, so the suffix sum becomes a forward prefix scan run by
tensor_tensor_scan directly out of PSUM, with the host-computed
beyond-core carry as the scan's initial.  out[:, q'] = sfx[:, q'] *
(1/count) on Pool; the one-column shift converts inclusive to exclusive
suffix sums.

Pipeline: per-tile chains (8 matmuls -> scan -> scale -> DMA) in
jt-outer order with a 4-buffer PSUM pool, so tiles drain progressively
and the scan/scale engines stay paced.  Input DMA delivers the vt chunks
for the first tile in k order, so the first tile's k-chain crawls with
the arrivals while dummy matmuls warm the PE (HAM) from kernel start;
every later tile runs at full PE rate.  The final tile drains in a
384+128 split so the closing scan->scale->DMA chain is short.

Sharding: 8 cores = 4 batches x 2 sequence halves; each core owns 1024
output rows exclusively (no partial sums).  Data path runs in fp16 with
fp32 PSUM/scan accumulation; rows with no masked entry (only the global
last row) get a true softmax, patched on the host from the raw inputs.
"""

import numpy as np

import concourse.bass as bass
import concourse.mybir as mybir
import concourse.tile as tile
from concourse import bacc, bass_utils

# ---------------------------------------------------------------- constants
B, S, D = 4, 2048, 1024
HEADS, DK = 16, 64
N_CORES = 8
SH = S // 2                 # 1024 sequence rows per core
NJT = D // 128              # 8 output-row (d_out) tiles
NK = D // 128               # 8 contraction chunks
NQC = 2                     # two 512-wide q' column tiles
CW = 512
MASK_CONST = np.float32(4294967296.0)   # +2^32, faithful to the reference
SCALE = 1.0 / np.sqrt(np.float32(DK))   # 1/8

F32 = mybir.dt.float32
FP16 = mybir.dt.float16
ALU = mybir.AluOpType


# ------------------------------------------------------------- kernel build
def _build():
    nc = bacc.Bacc("TRN2", target_bir_lowering=False, debug=False,
                   num_devices=N_CORES)

    def din(name, shape, dt):
        return nc.dram_tensor(name, shape, dt, kind="ExternalInput").ap()

    # g[jt][p_d, k, j_in] = G[k*128+p_d, jt*128+j_in]
    g = din("g", (NJT, 128, NK, 128), FP16)
    # vt[qc*4+kp][p_d, k2, q'] = values_rev[qc*512+q', (kp*2+k2)*128+p_d]
    vt = din("vt", (NQC * 4, 128, NK // 4, CW), FP16)
    # rtot[p, jt] = (sum of values rows beyond this core) @ G[:, jt*128+p]
    rtot = din("rtot", (128, NJT), F32)
    # sbc[j, q'] = 1/count in reversed order (0 at count==0), broadcast over j
    sbcd = din("sbc", (128, SH), FP16)

    out = nc.dram_tensor("out", (D, SH), FP16, kind="ExternalOutput").ap()
    wout = nc.dram_tensor("wout", (128, 16), FP16, kind="ExternalOutput").ap()

    with tile.TileContext(nc) as tc:
        with (
            tc.tile_pool(name="res", bufs=1) as res,
            tc.tile_pool(name="osb", bufs=3) as osb,
            tc.tile_pool(name="vgps", bufs=4, space="PSUM") as vgps,
        ):
            g_sb = res.tile([128, NJT, NK, 128], FP16, tag="g")
            vt_sb = res.tile([128, NQC, NK, CW], FP16, tag="vt")
            rtot_sb = res.tile([128, NJT], F32, tag="rtot")
            sbc = res.tile([128, SH], FP16, tag="sbc")
            zeros = res.tile([128, CW], F32, tag="zeros")
            sfx = res.tile([128, NJT, 1 + SH], F32, tag="sfx")
            scr = res.tile([128, CW], FP16, tag="scr")
            warm = res.tile([128, 16], FP16, tag="warm")

            # constants on Pool (free early; DVE stays clear for scans)
            nc.gpsimd.memset(zeros[:], 0.0)
            nc.gpsimd.memset(scr[:], 0.125)

            # ---- input DMAs.  sync: vt k-pairs in k order (first tile's
            # chain consumes them as they land), then qc1.  scalar: g in
            # jt order interleaved with the small scan/scale inputs.
            def vt_dma(qc, kp):
                nc.sync.dma_start(vt_sb[:, qc, kp * 2:(kp + 1) * 2, :],
                                  vt[qc * 4 + kp])

            for kp in range(4):
                vt_dma(0, kp)
            for kp in range(4):
                vt_dma(1, kp)

            nc.scalar.dma_start(g_sb[:, 0], g[0])
            nc.scalar.dma_start(g_sb[:, 1], g[1])
            nc.scalar.dma_start(rtot_sb[:], rtot[:])
            nc.scalar.dma_start(sbc[:], sbcd[:])
            for jt in range(2, NJT):
                nc.scalar.dma_start(g_sb[:, jt], g[jt])

            # ---- PE warm-up while the first chunks land (HAM to 8/8)
            wps = vgps.tile([128, CW], F32, tag="vg512")
            for d in range(5):
                nc.tensor.matmul(wps[:], scr[:, 0:128], scr[:],
                                 start=(d == 0), stop=(d == 4))
            nc.scalar.copy(warm[:], wps[:, 0:16])
            nc.scalar.dma_start(wout[:], warm[:])

            # ---- per-tile chains: 8 matmuls -> DVE scan -> Pool scale ->
            # DMA.  jt-outer; PSUM pool (4 bufs) paces the drains.
            def emit_gemm(qc, jt):
                pst = vgps.tile([128, CW], F32, tag="vg512")
                for k in range(NK):
                    nc.tensor.matmul(pst[:], g_sb[:, jt, k, :],
                                     vt_sb[:, qc, k, :],
                                     start=(k == 0), stop=(k == NK - 1))
                return pst

            def emit_scan(qc, jt, ps, lo, w):
                base = qc * CW + lo
                if base == 0:
                    nc.vector.tensor_copy(sfx[:, jt, 0:1],
                                          rtot_sb[:, jt:jt + 1])
                    init = rtot_sb[:, jt:jt + 1]
                else:
                    init = sfx[:, jt, base:base + 1]
                nc.vector.tensor_tensor_scan(
                    sfx[:, jt, 1 + base:1 + base + w],
                    zeros[:, 0:w], ps[:, lo:lo + w], init, ALU.add, ALU.add)

            def emit_out(qc, jt, lo, w, mul_eng, dma_eng):
                base = qc * CW + lo
                ob = osb.tile([128, w], FP16, tag=f"ob{w}")
                mul_eng.tensor_mul(
                    ob[:], sfx[:, jt, base:base + w], sbc[:, base:base + w])
                dma_eng.dma_start(
                    out[jt * 128:(jt + 1) * 128, base:base + w], ob[:])

            for qc in range(NQC):
                for jt in range(NJT):
                    ps = emit_gemm(qc, jt)
                    if qc == NQC - 1 and jt == NJT - 1:
                        # short closing chain: 384 then 128, scale of the
                        # last chunk on DVE (no cross-engine handoff)
                        emit_scan(qc, jt, ps, 0, 384)
                        emit_out(qc, jt, 0, 384, nc.gpsimd, nc.scalar)
                        emit_scan(qc, jt, ps, 384, 128)
                        emit_out(qc, jt, 384, 128, nc.vector, nc.sync)
                    else:
                        emit_scan(qc, jt, ps, 0, CW)
                        eng = nc.sync if (qc * NJT + jt) % 2 else nc.scalar
                        emit_out(qc, jt, 0, CW, nc.gpsimd, eng)

    nc.compile()
    return nc


# ------------------------------------------------------------- host wrapper
_CACHE: dict = {}
LAST_RESULTS = None
LAST_IN_MAPS = None


def _get_kernel():
    if "k" not in _CACHE:
        _CACHE["k"] = _build()
    return _CACHE["k"]


def _host_fallback(values, mask2d, G, row_bias, out):
    """Generic-mask path (never hit for the causal-complement mask):
    P = indicator/row_count computed densely on the host."""
    ind = ((mask2d * MASK_CONST) == MASK_CONST).astype(np.float32)
    cnt = ind.sum(axis=1)
    ok = cnt > 0
    P = ind[ok] / cnt[ok, None]
    for b in range(B):
        out[b][ok] = (P @ values[b]) @ G + row_bias


def kernel(queries, keys, values, mask, Wq, bq, Wk, bk, Wv, bv, Wo, bo):
    queries = np.asarray(queries, dtype=np.float32)
    keys = np.asarray(keys, dtype=np.float32)
    values = np.asarray(values, dtype=np.float32)
    mask2d = np.ascontiguousarray(
        np.asarray(mask, dtype=np.float32).reshape(S, S))
    Wq = np.asarray(Wq, dtype=np.float32); bq_ = np.asarray(bq, dtype=np.float32)
    Wk = np.asarray(Wk, dtype=np.float32); bk_ = np.asarray(bk, dtype=np.float32)
    Wv = np.asarray(Wv, dtype=np.float32); bv_ = np.asarray(bv, dtype=np.float32)
    Wo = np.asarray(Wo, dtype=np.float32); bo_ = np.asarray(bo, dtype=np.float32)

    G = Wv @ Wo                                  # (D, D) fp32
    row_bias = bv_ @ Wo + bo_                    # (D,)

    ind = ((mask2d * MASK_CONST) == MASK_CONST)
    qfix = np.where(~ind.any(axis=1))[0]
    causal = np.array_equal(
        ind, np.triu(np.ones((S, S), dtype=bool), k=1))

    out = np.empty((B, S, D), dtype=np.float32)

    if causal:
        nc = _get_kernel()

        G16 = G.astype(np.float16)
        g_host = np.ascontiguousarray(
            G16.reshape(NK, 128, NJT, 128).transpose(2, 1, 0, 3))

        # count(global q) = 2047 - q; reversed per-core
        counts = (S - 1) - np.arange(S, dtype=np.float64)
        counts[S - 1] = 1.0
        inv_cnt = (1.0 / counts).astype(np.float32)
        inv_cnt[S - 1] = 0.0

        in_maps = []
        for core in range(N_CORES):
            b, h = divmod(core, 2)
            vhalf_rev = values[b, h * SH:(h + 1) * SH, :][::-1].astype(
                np.float16)
            vt_host = np.ascontiguousarray(
                vhalf_rev.reshape(NQC, CW, 4, 2, 128)
                .transpose(0, 2, 4, 3, 1)).reshape(NQC * 4, 128, NK // 4, CW)
            if h == 0:
                beyond = values[b, SH:, :].sum(axis=0, dtype=np.float64)
                rtot_vec = (beyond.astype(np.float32) @ G)
            else:
                rtot_vec = np.zeros(D, dtype=np.float32)
            rtot_host = np.ascontiguousarray(rtot_vec.reshape(NJT, 128).T)
            sbc_host = np.ascontiguousarray(np.broadcast_to(
                inv_cnt[h * SH:(h + 1) * SH][::-1].astype(np.float16),
                (128, SH)))
            in_maps.append({
                "g": g_host,
                "vt": vt_host,
                "rtot": rtot_host,
                "sbc": sbc_host,
            })

        res = bass_utils.run_bass_kernel_spmd(
            nc, in_maps, core_ids=list(range(N_CORES)))

        global LAST_RESULTS, LAST_IN_MAPS
        LAST_RESULTS = res
        LAST_IN_MAPS = in_maps

        for core in range(N_CORES):
            b, h = divmod(core, 2)
            # out dram is [d_out, q'] with q' reversed: undo both
            o = res.results[core]["out"].astype(np.float32).T[::-1, :]
            out[b, h * SH:(h + 1) * SH, :] = o + row_bias
    else:
        _host_fallback(values, mask2d, G, row_bias, out)

    # ---------------- host patch for rows with no indicator entry
    # True softmax for these rows, by reassociation so neither Q nor K is
    # ever materialized: s = ((q Wq) Wk^T) keys^T; pure fp32 numpy.
    if len(qfix) > 0:
        q = qfix
        mrow = mask2d[q] * MASK_CONST                       # [nq, S]
        for b in range(B):
            Qr = queries[b][q] @ Wq + bq_                   # [nq, HEADS*DK]
            Oc = np.empty((len(q), HEADS * DK), dtype=np.float32)
            for H in range(HEADS):
                hs = slice(H * DK, (H + 1) * DK)
                t = Qr[:, hs] @ Wk[:, hs].T                 # [nq, D]
                scr = t @ keys[b].T                         # [nq, S]
                scr = scr + (Qr[:, hs] @ bk_[hs])[:, None]  # K-bias term
                y = (scr + mrow) * np.float32(SCALE)
                y = y - y.max(axis=1, keepdims=True)
                e = np.exp(y, dtype=np.float32)
                p = (e / e.sum(axis=1, keepdims=True)).astype(np.float32)
                z = p @ values[b]                           # [nq, D]
                Oc[:, hs] = z @ Wv[:, hs] + bv_[hs]
            out[b][q] = Oc @ Wo + bo_
    return out.reshape(B, S, D)


# revision 14
# speedup vs baseline: 1.3725x; 1.0535x over previous
"""Trainium2 Bass kernel for the 16-head MHA problem (B=4, S=2048, D=1024).

The reference adds mask*2^32 to the raw scores BEFORE the 1/sqrt(dk) scale
and softmax.  In fp32, for any row with at least one entry where
fl32(mask*2^32) == 2^32, the masked softmax collapses exactly to
indicator/row_count, identical for all 16 heads.  With G = Wv @ Wo
precomputed from the weight inputs the whole module factors:

    out[b] = (P @ values[b]) @ G + (bv @ Wo + bo)

For the causal-complement mask P@x is a suffix-mean.  Per core the device
work is a dense GEMM VG = values^T projected through G (output-transposed,
[d_out, seq] layout) plus a DVE prefix scan: the host packs the sequence
axis REVERSED, so the suffix sum becomes a forward prefix scan run by
tensor_tensor_scan directly out of PSUM, with the host-computed
beyond-core carry as the scan's initial.  out[:, q'] = sfx[:, q'] *
(1/count) on Pool; the one-column shift converts inclusive to exclusive
suffix sums.

Pipeline: per-tile chains (8 matmuls -> scan -> scale -> DMA) in
jt-outer order with a 4-buffer PSUM pool, so tiles drain progressively
and the scan/scale engines stay paced.  Input DMA delivers the vt chunks
for the first tile in k order, so the first tile's k-chain crawls with
the arrivals while dummy matmuls warm the PE (HAM) from kernel start;
every later tile runs at full PE rate.  The final tile drains in a
384+128 split so the closing scan->scale->DMA chain is short.

Sharding: 8 cores = 4 batches x 2 sequence halves; each core owns 1024
output rows exclusively (no partial sums).  Data path runs in fp16 with
fp32 PSUM/scan accumulation; rows with no masked entry (only the global
last row) get a true softmax, patched on the host from the raw inputs.
"""

import numpy as np

import concourse.bass as bass
import concourse.mybir as mybir
import concourse.tile as tile
from concourse import bacc, bass_utils

# ---------------------------------------------------------------- constants
B, S, D = 4, 2048, 1024
HEADS, DK = 16, 64
N_CORES = 8
SH = S // 2                 # 1024 sequence rows per core
NJT = D // 128              # 8 output-row (d_out) tiles
NK = D // 128               # 8 contraction chunks
NQC = 2                     # two 512-wide q' column tiles
CW = 512
MASK_CONST = np.float32(4294967296.0)   # +2^32, faithful to the reference
SCALE = 1.0 / np.sqrt(np.float32(DK))   # 1/8

F32 = mybir.dt.float32
FP16 = mybir.dt.float16
ALU = mybir.AluOpType


# ------------------------------------------------------------- kernel build
def _build():
    nc = bacc.Bacc("TRN2", target_bir_lowering=False, debug=False,
                   num_devices=N_CORES)

    def din(name, shape, dt):
        return nc.dram_tensor(name, shape, dt, kind="ExternalInput").ap()

    # g[jt][p_d, k, j_in] = G[k*128+p_d, jt*128+j_in]
    g = din("g", (NJT, 128, NK, 128), FP16)
    # vt[qc*4+kp][p_d, k2, q'] = values_rev[qc*512+q', (kp*2+k2)*128+p_d]
    vt = din("vt", (NQC * 4, 128, NK // 4, CW), FP16)
    # rtot[p, jt] = (sum of values rows beyond this core) @ G[:, jt*128+p]
    rtot = din("rtot", (128, NJT), F32)
    # sbc[j, q'] = 1/count in reversed order (0 at count==0), broadcast over j
    sbcd = din("sbc", (128, SH), FP16)

    out = nc.dram_tensor("out", (D, SH), FP16, kind="ExternalOutput").ap()
    wout = nc.dram_tensor("wout", (128, 16), FP16, kind="ExternalOutput").ap()

    with tile.TileContext(nc) as tc:
        with (
            tc.tile_pool(name="res", bufs=1) as res,
            tc.tile_pool(name="osb", bufs=3) as osb,
            tc.tile_pool(name="vgps", bufs=4, space="PSUM") as vgps,
            tc.tile_pool(name="vgps_s", bufs=2, space="PSUM") as vgps_s,
        ):
            g_sb = res.tile([128, NJT, NK, 128], FP16, tag="g")
            vt_sb = res.tile([128, NQC, NK, CW], FP16, tag="vt")
            rtot_sb = res.tile([128, NJT], F32, tag="rtot")
            sbc = res.tile([128, SH], FP16, tag="sbc")
            zeros = res.tile([128, CW], F32, tag="zeros")
            sfx = res.tile([128, NJT, 1 + SH], F32, tag="sfx")
            scr = res.tile([128, 256], FP16, tag="scr")
            warm = res.tile([128, 16], FP16, tag="warm")

            # constants on Pool (scr first: it gates the warm-up matmuls)
            nc.gpsimd.memset(scr[:], 0.125)
            nc.gpsimd.memset(zeros[:], 0.0)

            # ---- input DMAs, split across both queues so the first tile's
            # k-chain and the g stream land just ahead of consumption.
            def vt_dma(eng, qc, kp):
                eng.dma_start(vt_sb[:, qc, kp * 2:(kp + 1) * 2, :],
                              vt[qc * 4 + kp])

            vt_dma(nc.sync, 0, 0)
            vt_dma(nc.sync, 0, 1)
            vt_dma(nc.sync, 0, 3)
            for kp in range(4):
                vt_dma(nc.sync, 1, kp)

            nc.scalar.dma_start(g_sb[:, 0], g[0])
            vt_dma(nc.scalar, 0, 2)
            nc.scalar.dma_start(rtot_sb[:], rtot[:])
            nc.scalar.dma_start(g_sb[:, 1], g[1])
            nc.scalar.dma_start(g_sb[:, 2], g[2])
            nc.scalar.dma_start(sbc[:], sbcd[:])
            nc.scalar.dma_start(g_sb[:, 3], g[3])

            # ---- PE warm-up while the first chunks land (HAM to 8/8)
            wps = vgps.tile([128, CW], F32, tag="vg512")
            for d in range(14):
                nc.tensor.matmul(wps[:, 0:256], scr[:, 0:128], scr[:, 0:256],
                                 start=(d == 0), stop=(d == 13))
            nc.scalar.copy(warm[:], wps[:, 0:16])
            nc.scalar.dma_start(wout[:], warm[:])
            for jt in range(4, NJT):
                nc.scalar.dma_start(g_sb[:, jt], g[jt])

            # ---- per-tile chains: 8 matmuls -> DVE scan -> Pool scale ->
            # DMA.  jt-outer; PSUM pool (4 bufs) paces the drains.
            def emit_gemm(qc, jt, lo=0, w=CW):
                if w == CW:
                    pst = vgps.tile([128, CW], F32, tag="vg512")
                else:
                    pst = vgps_s.tile([128, w], F32, tag="vgs")
                for k in range(NK):
                    nc.tensor.matmul(pst[:], g_sb[:, jt, k, :],
                                     vt_sb[:, qc, k, lo:lo + w],
                                     start=(k == 0), stop=(k == NK - 1))
                return pst

            def emit_scan(qc, jt, ps, lo, w, pslo=None):
                base = qc * CW + lo
                if base == 0:
                    nc.vector.tensor_copy(sfx[:, jt, 0:1],
                                          rtot_sb[:, jt:jt + 1])
                    init = rtot_sb[:, jt:jt + 1]
                else:
                    init = sfx[:, jt, base:base + 1]
                pslo = lo if pslo is None else pslo
                nc.vector.tensor_tensor_scan(
                    sfx[:, jt, 1 + base:1 + base + w],
                    zeros[:, 0:w], ps[:, pslo:pslo + w], init,
                    ALU.add, ALU.add)

            def emit_out(qc, jt, lo, w, mul_eng, dma_eng):
                base = qc * CW + lo
                ob = osb.tile([128, w], FP16, tag=f"ob{w}")
                mul_eng.tensor_mul(
                    ob[:], sfx[:, jt, base:base + w], sbc[:, base:base + w])
                dma_eng.dma_start(
                    out[jt * 128:(jt + 1) * 128, base:base + w], ob[:])

            for qc in range(NQC):
                for jt in range(NJT):
                    if qc == NQC - 1 and jt == NJT - 1:
                        # final tile as two short 256-wide chains so the
                        # closing scan->scale->DMA path is brief; the last
                        # scale runs on DVE (no cross-engine handoff)
                        psa = emit_gemm(qc, jt, 0, 256)
                        emit_scan(qc, jt, psa, 0, 256, pslo=0)
                        emit_out(qc, jt, 0, 256, nc.gpsimd, nc.scalar)
                        psb = emit_gemm(qc, jt, 256, 256)
                        emit_scan(qc, jt, psb, 256, 256, pslo=0)
                        emit_out(qc, jt, 256, 256, nc.vector, nc.sync)
                    else:
                        ps = emit_gemm(qc, jt)
                        emit_scan(qc, jt, ps, 0, CW)
                        eng = nc.sync if (qc * NJT + jt) % 2 else nc.scalar
                        emit_out(qc, jt, 0, CW, nc.gpsimd, eng)

    nc.compile()
    return nc


# ------------------------------------------------------------- host wrapper
_CACHE: dict = {}
LAST_RESULTS = None
LAST_IN_MAPS = None


def _get_kernel():
    if "k" not in _CACHE:
        _CACHE["k"] = _build()
    return _CACHE["k"]


def _host_fallback(values, mask2d, G, row_bias, out):
    """Generic-mask path (never hit for the causal-complement mask):
    P = indicator/row_count computed densely on the host."""
    ind = ((mask2d * MASK_CONST) == MASK_CONST).astype(np.float32)
    cnt = ind.sum(axis=1)
    ok = cnt > 0
    P = ind[ok] / cnt[ok, None]
    for b in range(B):
        out[b][ok] = (P @ values[b]) @ G + row_bias


def kernel(queries, keys, values, mask, Wq, bq, Wk, bk, Wv, bv, Wo, bo):
    queries = np.asarray(queries, dtype=np.float32)
    keys = np.asarray(keys, dtype=np.float32)
    values = np.asarray(values, dtype=np.float32)
    mask2d = np.ascontiguousarray(
        np.asarray(mask, dtype=np.float32).reshape(S, S))
    Wq = np.asarray(Wq, dtype=np.float32); bq_ = np.asarray(bq, dtype=np.float32)
    Wk = np.asarray(Wk, dtype=np.float32); bk_ = np.asarray(bk, dtype=np.float32)
    Wv = np.asarray(Wv, dtype=np.float32); bv_ = np.asarray(bv, dtype=np.float32)
    Wo = np.asarray(Wo, dtype=np.float32); bo_ = np.asarray(bo, dtype=np.float32)

    G = Wv @ Wo                                  # (D, D) fp32
    row_bias = bv_ @ Wo + bo_                    # (D,)

    ind = ((mask2d * MASK_CONST) == MASK_CONST)
    qfix = np.where(~ind.any(axis=1))[0]
    causal = np.array_equal(
        ind, np.triu(np.ones((S, S), dtype=bool), k=1))

    out = np.empty((B, S, D), dtype=np.float32)

    if causal:
        nc = _get_kernel()

        G16 = G.astype(np.float16)
        g_host = np.ascontiguousarray(
            G16.reshape(NK, 128, NJT, 128).transpose(2, 1, 0, 3))

        # count(global q) = 2047 - q; reversed per-core
        counts = (S - 1) - np.arange(S, dtype=np.float64)
        counts[S - 1] = 1.0
        inv_cnt = (1.0 / counts).astype(np.float32)
        inv_cnt[S - 1] = 0.0

        in_maps = []
        for core in range(N_CORES):
            b, h = divmod(core, 2)
            vhalf_rev = values[b, h * SH:(h + 1) * SH, :][::-1].astype(
                np.float16)
            vt_host = np.ascontiguousarray(
                vhalf_rev.reshape(NQC, CW, 4, 2, 128)
                .transpose(0, 2, 4, 3, 1)).reshape(NQC * 4, 128, NK // 4, CW)
            if h == 0:
                beyond = values[b, SH:, :].sum(axis=0, dtype=np.float64)
                rtot_vec = (beyond.astype(np.float32) @ G)
            else:
                rtot_vec = np.zeros(D, dtype=np.float32)
            rtot_host = np.ascontiguousarray(rtot_vec.reshape(NJT, 128).T)
            sbc_host = np.ascontiguousarray(np.broadcast_to(
                inv_cnt[h * SH:(h + 1) * SH][::-1].astype(np.float16),
                (128, SH)))
            in_maps.append({
                "g": g_host,
                "vt": vt_host,
                "rtot": rtot_host,
                "sbc": sbc_host,
            })

        res = bass_utils.run_bass_kernel_spmd(
            nc, in_maps, core_ids=list(range(N_CORES)))

        global LAST_RESULTS, LAST_IN_MAPS
        LAST_RESULTS = res
        LAST_IN_MAPS = in_maps

        for core in range(N_CORES):
            b, h = divmod(core, 2)
            # out dram is [d_out, q'] with q' reversed: undo both
            o = res.results[core]["out"].astype(np.float32).T[::-1, :]
            out[b, h * SH:(h + 1) * SH, :] = o + row_bias
    else:
        _host_fallback(values, mask2d, G, row_bias, out)

    # ---------------- host patch for rows with no indicator entry
    # True softmax for these rows, by reassociation so neither Q nor K is
    # ever materialized: s = ((q Wq) Wk^T) keys^T; pure fp32 numpy.
    if len(qfix) > 0:
        q = qfix
        mrow = mask2d[q] * MASK_CONST                       # [nq, S]
        for b in range(B):
            Qr = queries[b][q] @ Wq + bq_                   # [nq, HEADS*DK]
            Oc = np.empty((len(q), HEADS * DK), dtype=np.float32)
            for H in range(HEADS):
                hs = slice(H * DK, (H + 1) * DK)
                t = Qr[:, hs] @ Wk[:, hs].T                 # [nq, D]
                scr = t @ keys[b].T                         # [nq, S]
                scr = scr + (Qr[:, hs] @ bk_[hs])[:, None]  # K-bias term
                y = (scr + mrow) * np.float32(SCALE)
                y = y - y.max(axis=1, keepdims=True)
                e = np.exp(y, dtype=np.float32)
                p = (e / e.sum(axis=1, keepdims=True)).astype(np.float32)
                z = p @ values[b]                           # [nq, D]
                Oc[:, hs] = z @ Wv[:, hs] + bv_[hs]
            out[b][q] = Oc @ Wo + bo_
    return out.reshape(B, S, D)
